# revision 19
# baseline (speedup 1.0000x reference)
"""Causal multi-head attention on 8 TRN2 NeuronCores.

Sharding: core c -> (batch b = c // 4, head-group g = c % 4, heads 4g..4g+3).
Each core computes its 4 heads' attention plus the partial output projection
(out_partial^T = W_O_g^T @ z_g^T, [1024, 2048] fp32). Host sums the 4 head-group
partials per batch, adds b_O, transposes back.

On-core layout ("T" = feature-major, seq on the free dim):
  x^T [1024, 2048]  (host-transposed, bf16)
  q^T/k^T [256, 2048] = 2 SBUF tiles [128, 2, 2048], head h at partitions
     (h%2)*64..(h%2)*64+63 of tile h//2
  scores^T tiles [k=128, q=512] via K=64 matmuls; even/odd heads use PE row
     groups 0-63 / 64-127 concurrently
  softmax: no max-subtraction needed (|scores|<~3), exp on ACT with scale=1/8,
     causal mask as multiplicative bf16 0/1 mask on the 4 diagonal tiles
  AV: v_aug [k=128, 65] stationary (col 64 = ones -> denominator in PSUM row 64),
     P^T moving; denominator -> ones-matmul broadcast -> reciprocal -> z^T
"""

import os
import sys

import numpy as np

for _p in ("/opt/trn_rl_repo", "/opt/pypackages"):
    if _p not in sys.path and os.path.isdir(_p):
        sys.path.append(_p)

import ml_dtypes  # noqa: E402

import concourse.bass as bass  # noqa: E402
import concourse.mybir as mybir  # noqa: E402
import concourse.tile as tile  # noqa: E402
from concourse import bacc  # noqa: E402
from concourse.bass_utils import run_bass_kernel_spmd  # noqa: E402

BF16 = mybir.dt.bfloat16
F32 = mybir.dt.float32
NPBF16 = ml_dtypes.bfloat16

B = 2
S = 2048
D = 1024
N_HEADS = 16
DH = 64
NH_CORE = 4          # heads per core
HE = NH_CORE * DH    # 256 concatenated head dims per core
QB = 512             # q block (moving operand width)
NQ = S // QB         # 4
NKT = S // 128       # 16 k-position tiles
NDT = D // 128       # 8 d_model tiles

LAST_RESULT = None
_GRAPH_CACHE = {}


def _emit(nc, tc, ctx, bias_qkv):
    import contextlib

    xT = nc.declare_dram_parameter("xT", [D, S], BF16, isOutput=False).ap()
    wq = nc.declare_dram_parameter("wq", [D, HE], BF16, isOutput=False).ap()
    wk = nc.declare_dram_parameter("wk", [D, HE], BF16, isOutput=False).ap()
    wv = nc.declare_dram_parameter("wv", [D, HE], BF16, isOutput=False).ap()
    wo = nc.declare_dram_parameter("wo", [HE, D], BF16, isOutput=False).ap()
    masks = nc.declare_dram_parameter("masks", [128, 4 * QB], BF16, isOutput=False).ap()
    if bias_qkv:
        bq = nc.declare_dram_parameter("bq", [HE], F32, isOutput=False).ap()
        bk = nc.declare_dram_parameter("bk", [HE], F32, isOutput=False).ap()
        bv = nc.declare_dram_parameter("bv", [HE], F32, isOutput=False).ap()
    out = nc.declare_dram_parameter("out", [D, S], F32, isOutput=True).ap()

    consts = ctx.enter_context(tc.tile_pool(name="consts", bufs=1))

    # Per-d-tile DMAs so the first QKV matmuls can start before the whole
    # input lands (the monolithic version showed a ~17us PE startup gap).
    xT_sb = consts.tile([128, NDT, S], BF16)
    wq_sb = consts.tile([128, NDT, HE], BF16)
    wk_sb = consts.tile([128, NDT, HE], BF16)
    wv_sb = consts.tile([128, NDT, HE], BF16)
    for t in range(NDT):
        rsl = slice(t * 128, (t + 1) * 128)
        nc.sync.dma_start(out=wq_sb[:, t, :], in_=wq[rsl, :])
        nc.sync.dma_start(out=wk_sb[:, t, :], in_=wk[rsl, :])
        nc.sync.dma_start(out=wv_sb[:, t, :], in_=wv[rsl, :])
        nc.sync.dma_start(out=xT_sb[:, t, :], in_=xT[rsl, :])
    wo_sb = consts.tile([128, 2, D], BF16)
    nc.sync.dma_start(out=wo_sb, in_=wo.rearrange("(t p) d -> p t d", p=128))
    mask_sb = consts.tile([128, 4 * QB], BF16)
    nc.sync.dma_start(out=mask_sb, in_=masks)

    ones_sb = consts.tile([128, DH], BF16)
    nc.vector.memset(ones_sb, 1.0)

    qT_sb = consts.tile([128, 2, S], BF16)
    kT_sb = consts.tile([128, 2, S], BF16)
    v_sb = consts.tile([128, NKT, NH_CORE, DH + 1], BF16)
    nc.vector.memset(v_sb, 1.0)  # col 64 of each head block stays 1.0 (ones row)
    zT_sb = consts.tile([128, 2, S], BF16)

    if bias_qkv:
        bq_sb = consts.tile([128, 2], F32)
        nc.sync.dma_start(out=bq_sb, in_=bq.rearrange("(t p) -> p t", p=128))
        bk_sb = consts.tile([128, 2], F32)
        nc.sync.dma_start(out=bk_sb, in_=bk.rearrange("(t p) -> p t", p=128))
        bv_r = bv.rearrange("(h e) -> h e", h=NH_CORE)
        bv_bcast = bass.AP(tensor=bv_r.tensor, offset=bv_r.offset,
                           ap=[[0, 128]] + list(bv_r.ap))
        bv_sb = consts.tile([128, NH_CORE, DH], F32)
        nc.sync.dma_start(out=bv_sb, in_=bv_bcast)

    # ---------------- QKV projections ----------------
    # v first (AV needs it from the first pipelined iteration), then q/k for
    # heads 0-1 only; q/k for heads 2-3 are emitted inside the attention loop
    # so those matmuls fill PE slack while ACT computes the first exps.
    qkvps = ctx.enter_context(tc.tile_pool(name="qkvps", bufs=1, space="PSUM"))

    def emit_v(vpool):
        for kt in range(NKT):
            ps = vpool.tile([128, HE], F32, tag="vproj", name=f"vps{kt}")
            for t in range(NDT):
                nc.tensor.matmul(
                    ps,
                    xT_sb[:, t, kt * 128:(kt + 1) * 128],
                    wv_sb[:, t, :],
                    start=(t == 0),
                    stop=(t == NDT - 1),
                )
            ps_v = ps.rearrange("p (h e) -> p h e", h=NH_CORE)
            if bias_qkv:
                nc.vector.tensor_add(v_sb[:, kt, :, 0:DH], ps_v, bv_sb)
            else:
                nc.vector.tensor_copy(out=v_sb[:, kt, :, 0:DH], in_=ps_v)

    def emit_qk(ht):
        for wsb, dst, bias_sb in (
            (wk_sb, kT_sb, "bk"),
            (wq_sb, qT_sb, "bq"),
        ):
            for qi in range(NQ):
                ps = qkvps.tile([128, QB], F32, tag="proj",
                                name=f"qkps{ht}{qi}{bias_sb}")
                for t in range(NDT):
                    nc.tensor.matmul(
                        ps,
                        wsb[:, t, ht * 128:(ht + 1) * 128],
                        xT_sb[:, t, qi * QB:(qi + 1) * QB],
                        start=(t == 0),
                        stop=(t == NDT - 1),
                    )
                dst_ap = dst[:, ht, qi * QB:(qi + 1) * QB]
                if bias_qkv:
                    bsb = bq_sb if bias_sb == "bq" else bk_sb
                    nc.scalar.activation(
                        out=dst_ap, in_=ps,
                        func=mybir.ActivationFunctionType.Identity,
                        bias=bsb[:, ht:ht + 1],
                    )
                else:
                    nc.vector.tensor_copy(out=dst_ap, in_=ps)

    with tc.tile_pool(name="vpool", bufs=2, space="PSUM") as vpool:
        emit_v(vpool)
    emit_qk(0)

    # ---------------- attention + output projection ----------------
    # PSUM: spool 2x[128,2,512] (4 banks) + zpool 3x[128,512] + qkvps 1 = 8
    spool = ctx.enter_context(tc.tile_pool(name="spool", bufs=2, space="PSUM"))
    zpool = ctx.enter_context(tc.tile_pool(name="zpool", bufs=3, space="PSUM"))
    # pT tiles of two consecutive (qi, ht) iterations are alive at once
    # (scores(i) produces while AV(i-1) consumes): up to 8 + 8 pairs.
    ppool = ctx.enter_context(tc.tile_pool(name="ppool", bufs=18))
    dpool = ctx.enter_context(tc.tile_pool(name="dpool", bufs=2))
    opool = ctx.enter_context(tc.tile_pool(name="opool", bufs=3))

    # Software-pipelined attention: iteration i = (qi, ht). Emit scores(i)
    # then AV(i-1): by the time PE reaches AV(i-1) in its in-order stream,
    # exp(i-1) (ACT) and masks(i-1) (GpSimd) finished during scores(i).
    # Row-pairing: per (pj, u), the even-head MM (partitions 0-63) and the
    # odd-head MM (64-127) are emitted back-to-back and run concurrently
    # in the PE array (measured 4ns start delta).
    def emit_scores(qi, ht):
        pTs = []
        for pj in range(2 * qi + 2):
            qoffs = [max(0, (2 * pj + u) - 4 * qi) * 128 for u in range(2)]
            ps_pair = [
                spool.tile([128, 2, QB], F32, tag="sc", name=f"sc{qi}{ht}{pj}a"),
                spool.tile([128, 2, QB], F32, tag="sc", name=f"sc{qi}{ht}{pj}b"),
            ]
            for u in range(2):
                kj = 2 * pj + u
                qo = qoffs[u]
                for hb in range(2):
                    pb = hb * 64
                    nc.tensor.matmul(
                        ps_pair[hb][:, u, qo:],
                        kT_sb[pb:pb + 64, ht, kj * 128:(kj + 1) * 128],
                        qT_sb[pb:pb + 64, ht, qi * QB + qo:(qi + 1) * QB],
                        start=True,
                        stop=True,
                    )
            pT_pair = [
                ppool.tile([128, 2, QB], BF16, tag="pT", name=f"pT{qi}{ht}{pj}a"),
                ppool.tile([128, 2, QB], BF16, tag="pT", name=f"pT{qi}{ht}{pj}b"),
            ]
            # One exp per (pair, head) over [:, :, min(qoffs):]. For diagonal
            # pairs this covers a slice of unwritten psum for the narrower u;
            # those values are bounded stale scores and never read (the AV
            # matmul reads only [qo_u:]).
            qo_pair = min(qoffs)
            for hb in range(2):
                nc.scalar.activation(
                    out=pT_pair[hb][:, :, qo_pair:], in_=ps_pair[hb][:, :, qo_pair:],
                    func=mybir.ActivationFunctionType.Exp,
                    scale=0.125,
                )
            for u in range(2):
                kj = 2 * pj + u
                if kj >= 4 * qi:  # diagonal tile: apply causal mask
                    j = kj - 4 * qi
                    qo = qoffs[u]
                    for hb in range(2):
                        nc.gpsimd.tensor_mul(
                            pT_pair[hb][:, u, qo:], pT_pair[hb][:, u, qo:],
                            mask_sb[:, j * QB + qo:(j + 1) * QB],
                        )
            pTs.append((pT_pair, qoffs))
        return pTs

    def emit_av(st):
        qi, ht, pTs = st["qi"], st["ht"], st["pTs"]
        nk = 4 * qi + 4
        qsl = slice(qi * QB, (qi + 1) * QB)
        zps = [
            zpool.tile([128, QB], F32, tag="ps1", name=f"zps{qi}{ht}a"),
            zpool.tile([128, QB], F32, tag="ps1", name=f"zps{qi}{ht}b"),
        ]
        for pj in range(nk // 2):
            pT_pair, qoffs = pTs[pj]
            for u in range(2):
                kj = 2 * pj + u
                qo = qoffs[u]
                for hb in range(2):
                    nc.tensor.matmul(
                        zps[hb][0:DH + 1, qo:],
                        v_sb[:, kj, 2 * ht + hb, :],
                        pT_pair[hb][:, u, qo:],
                        start=(kj == 0),
                        stop=(kj == nk - 1),
                    )
        for hb in range(2):
            dsb = dpool.tile([128, QB], BF16, tag="d")
            nc.vector.tensor_copy(out=dsb[DH:DH + 1, :], in_=zps[hb][DH:DH + 1, :])
            bps = qkvps.tile([DH, QB], F32, tag="proj",
                             name=f"bps{qi}{ht}{hb}")
            nc.tensor.matmul(
                bps,
                ones_sb[DH:DH + 1, :],
                dsb[DH:DH + 1, :],
                start=True,
                stop=True,
            )
            bsb = dpool.tile([DH, QB], F32, tag="bsb")
            nc.vector.reciprocal_approx_fast(bsb, bps)
            # direct partition-base-shifted write for the odd head (64-aligned
            # base shifts are ISA-legal)
            nc.vector.tensor_mul(
                zT_sb[hb * DH:(hb + 1) * DH, ht, qsl], zps[hb][0:DH, :], bsb)
        if ht == 1:
            for dt in range(NDT):
                ops = zpool.tile([128, QB], F32, tag="ps1")
                for t in range(2):
                    nc.tensor.matmul(
                        ops,
                        wo_sb[:, t, dt * 128:(dt + 1) * 128],
                        zT_sb[:, t, qsl],
                        start=(t == 0),
                        stop=(t == 1),
                    )
                osb = opool.tile([128, QB], F32, tag="ot")
                nc.vector.tensor_copy(out=osb, in_=ops)
                nc.sync.dma_start(out=out[dt * 128:(dt + 1) * 128, qsl], in_=osb)

    prev = None
    for it in range(8):
        # all ht=0 iterations first: heads 2-3 q/k projections (emitted after
        # the first scores batch) fill PE slack while ACT runs the early exps
        qi, ht = it % 4, it // 4
        pTs = emit_scores(qi, ht)
        if it == 0:
            emit_qk(1)
        if prev is not None:
            emit_av(prev)
        prev = {"qi": qi, "ht": ht, "pTs": pTs}
    emit_av(prev)


def _build(bias_qkv):
    key = bool(bias_qkv)
    if key in _GRAPH_CACHE:
        return _GRAPH_CACHE[key]
    import contextlib

    nc = bacc.Bacc("TRN2", target_bir_lowering=False, debug=False, num_devices=8)
    with contextlib.ExitStack() as ctx:
        tc = ctx.enter_context(tile.TileContext(nc))
        _emit(nc, tc, ctx, bias_qkv)
    nc.compile()
    _GRAPH_CACHE[key] = nc
    return nc


def _make_masks():
    kl = np.arange(128)[:, None]
    ql = np.arange(QB)[None, :]
    m = np.zeros((128, 4, QB), dtype=np.float32)
    for j in range(4):
        m[:, j, :] = (kl <= ql - 128 * j).astype(np.float32)
    return np.ascontiguousarray(m.reshape(128, 4 * QB)).astype(NPBF16)


def kernel(normalized_resid_pre, W_Q, W_K, W_V, W_O, b_Q, b_K, b_V, b_O):
    global LAST_RESULT
    x = np.asarray(normalized_resid_pre, dtype=np.float32)
    W_Q = np.asarray(W_Q, dtype=np.float32)
    W_K = np.asarray(W_K, dtype=np.float32)
    W_V = np.asarray(W_V, dtype=np.float32)
    W_O = np.asarray(W_O, dtype=np.float32)
    b_Q = np.asarray(b_Q, dtype=np.float32)
    b_K = np.asarray(b_K, dtype=np.float32)
    b_V = np.asarray(b_V, dtype=np.float32)
    b_O = np.asarray(b_O, dtype=np.float32)

    bias_qkv = bool(np.any(b_Q) or np.any(b_K) or np.any(b_V))
    nc = _build(bias_qkv)

    mask_np = _make_masks()
    xT = [np.ascontiguousarray(x[b].T).astype(NPBF16) for b in range(B)]

    in_maps = []
    for c in range(8):
        b, g = c // 4, c % 4
        hs = slice(4 * g, 4 * g + 4)
        m = {
            "xT": xT[b],
            "wq": np.ascontiguousarray(
                W_Q[hs].transpose(1, 0, 2).reshape(D, HE)).astype(NPBF16),
            "wk": np.ascontiguousarray(
                W_K[hs].transpose(1, 0, 2).reshape(D, HE)).astype(NPBF16),
            "wv": np.ascontiguousarray(
                W_V[hs].transpose(1, 0, 2).reshape(D, HE)).astype(NPBF16),
            "wo": np.ascontiguousarray(W_O[hs].reshape(HE, D)).astype(NPBF16),
            "masks": mask_np,
        }
        if bias_qkv:
            m["bq"] = np.ascontiguousarray(b_Q[hs].reshape(HE))
            m["bk"] = np.ascontiguousarray(b_K[hs].reshape(HE))
            m["bv"] = np.ascontiguousarray(b_V[hs].reshape(HE))
        in_maps.append(m)

    res = run_bass_kernel_spmd(nc, in_maps, list(range(8)))
    LAST_RESULT = res

    full = np.zeros((B, S, D), dtype=np.float32)
    for c in range(8):
        full[c // 4] += np.asarray(res.results[c]["out"], dtype=np.float32).T
    full += b_O[None, None, :]
    return full


# revision 21
# speedup vs baseline: 1.0585x; 1.0585x over previous
"""Causal multi-head attention on 8 TRN2 NeuronCores.

Sharding: core c -> (batch b = c // 4, head-group g = c % 4, heads 4g..4g+3).
Each core computes its 4 heads' attention plus the partial output projection
(out_partial^T = W_O_g^T @ z_g^T, [1024, 2048] fp32). Host sums the 4 head-group
partials per batch, adds b_O, transposes back.

On-core layout ("T" = feature-major, seq on the free dim):
  x^T [1024, 2048]  (host-transposed, bf16)
  q^T/k^T [256, 2048] = 2 SBUF tiles [128, 2, 2048], head h at partitions
     (h%2)*64..(h%2)*64+63 of tile h//2
  scores^T tiles [k=128, q=512] via K=64 matmuls; even/odd heads use PE row
     groups 0-63 / 64-127 concurrently
  softmax: no max-subtraction needed (|scores|<~3), exp on ACT with scale=1/8,
     causal mask as multiplicative bf16 0/1 mask on the 4 diagonal tiles
  AV: v_aug [k=128, 65] stationary (col 64 = ones -> denominator in PSUM row 64),
     P^T moving; denominator -> ones-matmul broadcast -> reciprocal -> z^T
"""

import os
import sys

import numpy as np

for _p in ("/opt/trn_rl_repo", "/opt/pypackages"):
    if _p not in sys.path and os.path.isdir(_p):
        sys.path.append(_p)

import ml_dtypes  # noqa: E402

import concourse.bass as bass  # noqa: E402
import concourse.mybir as mybir  # noqa: E402
import concourse.tile as tile  # noqa: E402
from concourse import bacc  # noqa: E402
from concourse.bass_utils import run_bass_kernel_spmd  # noqa: E402

BF16 = mybir.dt.bfloat16
F32 = mybir.dt.float32
NPBF16 = ml_dtypes.bfloat16

B = 2
S = 2048
D = 1024
N_HEADS = 16
DH = 64
NH_CORE = 4          # heads per core
HE = NH_CORE * DH    # 256 concatenated head dims per core
QB = 512             # q block (moving operand width)
NQ = S // QB         # 4
NKT = S // 128       # 16 k-position tiles
NDT = D // 128       # 8 d_model tiles

LAST_RESULT = None
_GRAPH_CACHE = {}


def _emit(nc, tc, ctx, bias_qkv):
    import contextlib

    xT = nc.declare_dram_parameter("xT", [D, S], BF16, isOutput=False).ap()
    wq = nc.declare_dram_parameter("wq", [D, HE], BF16, isOutput=False).ap()
    wk = nc.declare_dram_parameter("wk", [D, HE], BF16, isOutput=False).ap()
    wv = nc.declare_dram_parameter("wv", [D, HE], BF16, isOutput=False).ap()
    wo = nc.declare_dram_parameter("wo", [HE, D], BF16, isOutput=False).ap()
    masks = nc.declare_dram_parameter("masks", [128, 4 * QB], BF16, isOutput=False).ap()
    if bias_qkv:
        bq = nc.declare_dram_parameter("bq", [HE], F32, isOutput=False).ap()
        bk = nc.declare_dram_parameter("bk", [HE], F32, isOutput=False).ap()
        bv = nc.declare_dram_parameter("bv", [HE], F32, isOutput=False).ap()
    out = nc.declare_dram_parameter("out", [D, S], F32, isOutput=True).ap()

    consts = ctx.enter_context(tc.tile_pool(name="consts", bufs=1))

    # Per-d-tile DMAs so the first QKV matmuls can start before the whole
    # input lands (the monolithic version showed a ~17us PE startup gap).
    xT_sb = consts.tile([128, NDT, S], BF16)
    wq_sb = consts.tile([128, NDT, HE], BF16)
    wk_sb = consts.tile([128, NDT, HE], BF16)
    wv_sb = consts.tile([128, NDT, HE], BF16)
    for t in range(NDT):
        rsl = slice(t * 128, (t + 1) * 128)
        nc.sync.dma_start(out=wq_sb[:, t, :], in_=wq[rsl, :])
        nc.sync.dma_start(out=wk_sb[:, t, :], in_=wk[rsl, :])
        nc.sync.dma_start(out=wv_sb[:, t, :], in_=wv[rsl, :])
        nc.sync.dma_start(out=xT_sb[:, t, :], in_=xT[rsl, :])
    wo_sb = consts.tile([128, 2, D], BF16)
    nc.sync.dma_start(out=wo_sb, in_=wo.rearrange("(t p) d -> p t d", p=128))
    mask_sb = consts.tile([128, 4 * QB], BF16)
    nc.sync.dma_start(out=mask_sb, in_=masks)

    ones_sb = consts.tile([128, DH], BF16)
    nc.vector.memset(ones_sb, 1.0)

    qT_sb = consts.tile([128, 2, S], BF16)
    kT_sb = consts.tile([128, 2, S], BF16)
    v_sb = consts.tile([128, NKT, NH_CORE, DH + 1], BF16)
    nc.vector.memset(v_sb, 1.0)  # col 64 of each head block stays 1.0 (ones row)
    zT_sb = consts.tile([128, 2, S], BF16)

    if bias_qkv:
        bq_sb = consts.tile([128, 2], F32)
        nc.sync.dma_start(out=bq_sb, in_=bq.rearrange("(t p) -> p t", p=128))
        bk_sb = consts.tile([128, 2], F32)
        nc.sync.dma_start(out=bk_sb, in_=bk.rearrange("(t p) -> p t", p=128))
        bv_r = bv.rearrange("(h e) -> h e", h=NH_CORE)
        bv_bcast = bass.AP(tensor=bv_r.tensor, offset=bv_r.offset,
                           ap=[[0, 128]] + list(bv_r.ap))
        bv_sb = consts.tile([128, NH_CORE, DH], F32)
        nc.sync.dma_start(out=bv_sb, in_=bv_bcast)

    # ---------------- QKV projections ----------------
    # v first (AV needs it from the first pipelined iteration), then q/k for
    # heads 0-1 only; q/k for heads 2-3 are emitted inside the attention loop
    # so those matmuls fill PE slack while ACT computes the first exps.
    qkvps = ctx.enter_context(tc.tile_pool(name="qkvps", bufs=1, space="PSUM"))

    def emit_v(vpool):
        for kt in range(NKT):
            ps = vpool.tile([128, HE], F32, tag="vproj", name=f"vps{kt}")
            for t in range(NDT):
                nc.tensor.matmul(
                    ps,
                    xT_sb[:, t, kt * 128:(kt + 1) * 128],
                    wv_sb[:, t, :],
                    start=(t == 0),
                    stop=(t == NDT - 1),
                )
            ps_v = ps.rearrange("p (h e) -> p h e", h=NH_CORE)
            if bias_qkv:
                nc.vector.tensor_add(v_sb[:, kt, :, 0:DH], ps_v, bv_sb)
            else:
                nc.vector.tensor_copy(out=v_sb[:, kt, :, 0:DH], in_=ps_v)

    def emit_qk(ht):
        for wsb, dst, bias_sb in (
            (wk_sb, kT_sb, "bk"),
            (wq_sb, qT_sb, "bq"),
        ):
            for qi in range(NQ):
                ps = qkvps.tile([128, QB], F32, tag="proj",
                                name=f"qkps{ht}{qi}{bias_sb}")
                for t in range(NDT):
                    nc.tensor.matmul(
                        ps,
                        wsb[:, t, ht * 128:(ht + 1) * 128],
                        xT_sb[:, t, qi * QB:(qi + 1) * QB],
                        start=(t == 0),
                        stop=(t == NDT - 1),
                    )
                dst_ap = dst[:, ht, qi * QB:(qi + 1) * QB]
                if bias_qkv:
                    bsb = bq_sb if bias_sb == "bq" else bk_sb
                    nc.scalar.activation(
                        out=dst_ap, in_=ps,
                        func=mybir.ActivationFunctionType.Identity,
                        bias=bsb[:, ht:ht + 1],
                    )
                else:
                    nc.vector.tensor_copy(out=dst_ap, in_=ps)

    with tc.tile_pool(name="vpool", bufs=2, space="PSUM") as vpool:
        emit_v(vpool)
    emit_qk(0)

    # ---------------- attention + output projection ----------------
    # PSUM: spool 2x[128,2,512] (4 banks) + zpool 3x[128,512] + qkvps 1 = 8
    spool = ctx.enter_context(tc.tile_pool(name="spool", bufs=2, space="PSUM"))
    zpool = ctx.enter_context(tc.tile_pool(name="zpool", bufs=3, space="PSUM"))
    # pT tiles of two consecutive (qi, ht) iterations are alive at once
    # (scores(i) produces while AV(i-1) consumes): up to 8 + 8 pairs.
    ppool = ctx.enter_context(tc.tile_pool(name="ppool", bufs=18))
    dpool = ctx.enter_context(tc.tile_pool(name="dpool", bufs=2))
    opool = ctx.enter_context(tc.tile_pool(name="opool", bufs=3))

    # Software-pipelined attention: iteration i = (qi, ht). Emit scores(i)
    # then AV(i-1): by the time PE reaches AV(i-1) in its in-order stream,
    # exp(i-1) (ACT) and masks(i-1) (GpSimd) finished during scores(i).
    # Row-pairing: per (pj, u), the even-head MM (partitions 0-63) and the
    # odd-head MM (64-127) are emitted back-to-back and run concurrently
    # in the PE array (measured 4ns start delta).
    def emit_scores(qi, ht):
        pTs = []
        for pj in range(2 * qi + 2):
            qoffs = [max(0, (2 * pj + u) - 4 * qi) * 128 for u in range(2)]
            ps_pair = [
                spool.tile([128, 2, QB], F32, tag="sc", name=f"sc{qi}{ht}{pj}a"),
                spool.tile([128, 2, QB], F32, tag="sc", name=f"sc{qi}{ht}{pj}b"),
            ]
            for u in range(2):
                kj = 2 * pj + u
                qo = qoffs[u]
                for hb in range(2):
                    pb = hb * 64
                    nc.tensor.matmul(
                        ps_pair[hb][:, u, qo:],
                        kT_sb[pb:pb + 64, ht, kj * 128:(kj + 1) * 128],
                        qT_sb[pb:pb + 64, ht, qi * QB + qo:(qi + 1) * QB],
                        start=True,
                        stop=True,
                    )
            pT_pair = [
                ppool.tile([128, 2, QB], BF16, tag="pT", name=f"pT{qi}{ht}{pj}a"),
                ppool.tile([128, 2, QB], BF16, tag="pT", name=f"pT{qi}{ht}{pj}b"),
            ]
            # One exp per (pair, head) over [:, :, min(qoffs):]. For diagonal
            # pairs this covers a slice of unwritten psum for the narrower u;
            # those values are bounded stale scores and never read (the AV
            # matmul reads only [qo_u:]).
            qo_pair = min(qoffs)
            for hb in range(2):
                nc.scalar.activation(
                    out=pT_pair[hb][:, :, qo_pair:], in_=ps_pair[hb][:, :, qo_pair:],
                    func=mybir.ActivationFunctionType.Exp,
                    scale=0.125,
                )
            for u in range(2):
                kj = 2 * pj + u
                if kj >= 4 * qi:  # diagonal tile: apply causal mask
                    j = kj - 4 * qi
                    qo = qoffs[u]
                    for hb in range(2):
                        nc.gpsimd.tensor_mul(
                            pT_pair[hb][:, u, qo:], pT_pair[hb][:, u, qo:],
                            mask_sb[:, j * QB + qo:(j + 1) * QB],
                        )
            pTs.append((pT_pair, qoffs))
        return pTs

    def emit_av(st):
        qi, ht, pTs = st["qi"], st["ht"], st["pTs"]
        nk = 4 * qi + 4
        qsl = slice(qi * QB, (qi + 1) * QB)
        zps = [
            zpool.tile([128, QB], F32, tag="ps1", name=f"zps{qi}{ht}a"),
            zpool.tile([128, QB], F32, tag="ps1", name=f"zps{qi}{ht}b"),
        ]
        for pj in range(nk // 2):
            pT_pair, qoffs = pTs[pj]
            for u in range(2):
                kj = 2 * pj + u
                qo = qoffs[u]
                for hb in range(2):
                    nc.tensor.matmul(
                        zps[hb][0:DH + 1, qo:],
                        v_sb[:, kj, 2 * ht + hb, :],
                        pT_pair[hb][:, u, qo:],
                        start=(kj == 0),
                        stop=(kj == nk - 1),
                    )
        for hb in range(2):
            dsb = dpool.tile([128, QB], BF16, tag="d")
            nc.vector.tensor_copy(out=dsb[DH:DH + 1, :], in_=zps[hb][DH:DH + 1, :])
            bps = zpool.tile([DH, QB], F32, tag="ps1", name=f"bps{qi}{ht}{hb}")
            nc.tensor.matmul(
                bps,
                ones_sb[DH:DH + 1, :],
                dsb[DH:DH + 1, :],
                start=True,
                stop=True,
            )
            bsb = dpool.tile([DH, QB], F32, tag="bsb")
            nc.vector.reciprocal_approx_fast(bsb, bps)
            # direct partition-base-shifted write for the odd head (64-aligned
            # base shifts are ISA-legal)
            nc.vector.tensor_mul(
                zT_sb[hb * DH:(hb + 1) * DH, ht, qsl], zps[hb][0:DH, :], bsb)
        if ht == 1:
            for dt in range(NDT):
                ops = zpool.tile([128, QB], F32, tag="ps1")
                for t in range(2):
                    nc.tensor.matmul(
                        ops,
                        wo_sb[:, t, dt * 128:(dt + 1) * 128],
                        zT_sb[:, t, qsl],
                        start=(t == 0),
                        stop=(t == 1),
                    )
                osb = opool.tile([128, QB], F32, tag="ot")
                nc.vector.tensor_copy(out=osb, in_=ops)
                nc.sync.dma_start(out=out[dt * 128:(dt + 1) * 128, qsl], in_=osb)

    prev = None
    for it in range(8):
        # all ht=0 iterations first: heads 2-3 q/k projections (emitted after
        # the first scores batch) fill PE slack while ACT runs the early exps
        qi, ht = it % 4, it // 4
        pTs = emit_scores(qi, ht)
        if it == 0:
            emit_qk(1)
        if prev is not None:
            emit_av(prev)
        prev = {"qi": qi, "ht": ht, "pTs": pTs}
    emit_av(prev)


def _build(bias_qkv):
    key = bool(bias_qkv)
    if key in _GRAPH_CACHE:
        return _GRAPH_CACHE[key]
    import contextlib

    nc = bacc.Bacc("TRN2", target_bir_lowering=False, debug=False, num_devices=8)
    with contextlib.ExitStack() as ctx:
        tc = ctx.enter_context(tile.TileContext(nc))
        _emit(nc, tc, ctx, bias_qkv)
    nc.compile()
    _GRAPH_CACHE[key] = nc
    return nc


def _make_masks():
    kl = np.arange(128)[:, None]
    ql = np.arange(QB)[None, :]
    m = np.zeros((128, 4, QB), dtype=np.float32)
    for j in range(4):
        m[:, j, :] = (kl <= ql - 128 * j).astype(np.float32)
    return np.ascontiguousarray(m.reshape(128, 4 * QB)).astype(NPBF16)


def kernel(normalized_resid_pre, W_Q, W_K, W_V, W_O, b_Q, b_K, b_V, b_O):
    global LAST_RESULT
    x = np.asarray(normalized_resid_pre, dtype=np.float32)
    W_Q = np.asarray(W_Q, dtype=np.float32)
    W_K = np.asarray(W_K, dtype=np.float32)
    W_V = np.asarray(W_V, dtype=np.float32)
    W_O = np.asarray(W_O, dtype=np.float32)
    b_Q = np.asarray(b_Q, dtype=np.float32)
    b_K = np.asarray(b_K, dtype=np.float32)
    b_V = np.asarray(b_V, dtype=np.float32)
    b_O = np.asarray(b_O, dtype=np.float32)

    bias_qkv = bool(np.any(b_Q) or np.any(b_K) or np.any(b_V))
    nc = _build(bias_qkv)

    mask_np = _make_masks()
    xT = [np.ascontiguousarray(x[b].T).astype(NPBF16) for b in range(B)]

    in_maps = []
    for c in range(8):
        b, g = c // 4, c % 4
        hs = slice(4 * g, 4 * g + 4)
        m = {
            "xT": xT[b],
            "wq": np.ascontiguousarray(
                W_Q[hs].transpose(1, 0, 2).reshape(D, HE)).astype(NPBF16),
            "wk": np.ascontiguousarray(
                W_K[hs].transpose(1, 0, 2).reshape(D, HE)).astype(NPBF16),
            "wv": np.ascontiguousarray(
                W_V[hs].transpose(1, 0, 2).reshape(D, HE)).astype(NPBF16),
            "wo": np.ascontiguousarray(W_O[hs].reshape(HE, D)).astype(NPBF16),
            "masks": mask_np,
        }
        if bias_qkv:
            m["bq"] = np.ascontiguousarray(b_Q[hs].reshape(HE))
            m["bk"] = np.ascontiguousarray(b_K[hs].reshape(HE))
            m["bv"] = np.ascontiguousarray(b_V[hs].reshape(HE))
        in_maps.append(m)

    res = run_bass_kernel_spmd(nc, in_maps, list(range(8)))
    LAST_RESULT = res

    full = np.zeros((B, S, D), dtype=np.float32)
    for c in range(8):
        full[c // 4] += np.asarray(res.results[c]["out"], dtype=np.float32).T
    full += b_O[None, None, :]
    return full
